# revision 1
# baseline (speedup 1.0000x reference)
"""TransformerConv GNN block (nn_Block_28192165331060) on 8 Trainium2 NeuronCores.

Strategy (matches the sharding hint):
  - Nodes are sharded contiguously across the 8 cores (6250 each).
  - Edges are partitioned by destination-node owner; each core handles the
    segment softmax + aggregation for its own destination nodes.
  - k/v (and q) projection tables are materialized per-core in DRAM
    (replicated compute of k/v over the full node set instead of a halo
    all-gather - cheaper and collective-free).
  - Per-edge work is done in "windows": a window covers <=128 consecutive
    destination nodes and <=SLOT_CAP edge slots (host packs greedily, in
    destination order). Inside a window, edges are processed 128 at a time:
      * one indirect-DMA gather of the window's 128 q rows (Q_win), then
        per 128-edge tile: an indirect-DMA gather of kv rows (by src),
        a one-hot matrix O[e, n] = (rel_dst_e == n) (DVE is_equal vs iota),
        q_dst = O^T.T @ Q_win on the PE (permutation matmul - avoids a
        per-edge q gather), per-edge score = <q_dst, k>/sqrt(D) (DVE
        mult + reduce), p = exp(score/sqrt(D)) on ACT (no max subtraction
        needed: scores are O(1)), scat = O scaled by p (ACT per-partition
        scale), and a PE matmul scat^T @ [V | 1] accumulating [agg | den]
        in PSUM. Only 17 SWDGE instructions per window (the [128,1]-offset
        indirect DMA is the only gather primitive that works on this HW).
    The window result is indirect-scattered to an agg table (one row per
    node; padded rows go to per-window trash rows).
  - Post-attention dense math (skip proj, O proj, residuals, FFN) is done in
    transposed space [D, nodes] so every per-feature affine (BatchNorm, FFN
    biases) becomes a cheap per-partition scalar op on the ACT engine.
  - BatchNorm statistics are global: per-core partial (sum, sumsq) columns
    are AllReduce'd across the 8 cores (2 tiny collectives).
"""

import math

import numpy as np

N_NODES = 50000
D = 128
NC = 8
NL = N_NODES // NC          # 6250 nodes per core
NLP = 6272                  # padded local nodes (49 * 128)
NT_LOC = NLP // 128         # 49 local node tiles
NFULL = 50048               # padded full nodes (391 * 128)
NT_FULL = NFULL // 128      # 391
SLOT_CAP = 2048             # edge slots per window
TILES_PER_WIN = SLOT_CAP // 128   # 16
NW = 52                     # windows per core (compile-time)
TRASH0 = NLP + 256          # first trash row in agg table
AGG_ROWS = TRASH0 + NW * 128
EPS = 1e-5

F32 = None  # set lazily (mybir import)


# ---------------------------------------------------------------------------
# Host-side preprocessing
# ---------------------------------------------------------------------------

def _pack_windows(deg, slot_cap, max_nodes, nw_max):
    """Greedy pack of consecutive nodes into windows.

    Returns list of (base_node, n_nodes) per window covering [0, len(deg)).
    """
    wins = []
    base = 0
    n = len(deg)
    while base < n:
        used = 0
        cnt = 0
        while base + cnt < n and cnt < max_nodes:
            d = int(deg[base + cnt])
            if used + d > slot_cap:
                break
            used += d
            cnt += 1
        assert cnt > 0, "single node degree exceeds slot capacity"
        wins.append((base, cnt))
        base += cnt
    assert len(wins) <= nw_max, f"need {len(wins)} windows > {nw_max}"
    while len(wins) < nw_max:
        wins.append((0, 0))  # dummy window: all slots dummy, flush to trash
    return wins


def host_prep(x, edge_index, weights, cfg):
    """Build all per-core device input arrays.

    cfg: dict with keys n_nodes, nc, nl, nlp, nfull, slot_cap, nw, trash0
    weights: dict of the 18 parameter arrays (numpy float32)
    """
    n_nodes = cfg["n_nodes"]; nc_ = cfg["nc"]; nl = cfg["nl"]
    nlp = cfg["nlp"]; nfull = cfg["nfull"]; slot_cap = cfg["slot_cap"]
    nw = cfg["nw"]; trash0 = cfg["trash0"]
    tpw = slot_cap // 128

    x = np.asarray(x, dtype=np.float32)
    src = np.asarray(edge_index[0], dtype=np.int64)
    dst = np.asarray(edge_index[1], dtype=np.int64)

    W = {k: np.asarray(v, dtype=np.float32) for k, v in weights.items()}
    WsWO = (W["Ws"] @ W["WO"]).astype(np.float32)
    beff = (W["bs"] @ W["WO"] + W["bO"]).astype(np.float32)
    Wkv = np.concatenate([W["Wk"], W["Wv"]], axis=1).astype(np.float32)
    bkv_b = np.broadcast_to(
        np.concatenate([W["bk"], W["bv"]])[None, :], (128, 256)
    ).astype(np.float32).copy()
    bq_b = np.broadcast_to(W["bq"][None, :], (128, 128)).astype(np.float32).copy()

    # bias/affine columns: b1a, b1b, b2, g1, be1, g2, be2, pad
    bcols = np.zeros((128, 8), dtype=np.float32)
    bcols[:, 0] = W["b1"][0:128]
    bcols[:, 1] = W["b1"][128:256]
    bcols[:, 2] = W["b2"]
    bcols[:, 3] = W["g1"]
    bcols[:, 4] = W["be1"]
    bcols[:, 5] = W["g2"]
    bcols[:, 6] = W["be2"]

    x_full_pad = np.zeros((nfull, 128), dtype=np.float32)
    x_full_pad[:n_nodes] = x
    xT_full = np.ascontiguousarray(x_full_pad.T)

    shared = {
        "xT_full": xT_full,
        "Wkv": Wkv,
        "Wq_": W["Wq"].copy(),
        "bkv_b": bkv_b,
        "bq_b": bq_b,
        "WsWO": WsWO,
        "WO_": W["WO"].copy(),
        "W1_": W["W1"].copy(),
        "W2_": W["W2"].copy(),
        "bcols": bcols,
    }

    in_maps = []
    owner = dst // nl
    for c in range(nc_):
        lo = c * nl
        m = owner == c
        s_c = src[m]
        dl = (dst[m] - lo).astype(np.int64)
        order = np.argsort(dl, kind="stable")
        s_c = s_c[order]
        dl = dl[order]
        deg = np.bincount(dl, minlength=nlp).astype(np.int64)
        assert deg.max() <= slot_cap
        wins = _pack_windows(deg, slot_cap, 128, nw)

        # edge start offset of each node in the sorted edge list
        starts = np.zeros(nlp + 1, dtype=np.int64)
        np.cumsum(deg, out=starts[1:])

        meta = np.zeros((nw, 128, 49), dtype=np.int32)
        for w, (b, cnt) in enumerate(wins):
            # flush indices
            fl = np.full(128, trash0 + w * 128, dtype=np.int32) + np.arange(
                128, dtype=np.int32
            )
            if cnt > 0:
                fl[:cnt] = b + np.arange(cnt, dtype=np.int32)
            meta[w, :, 48] = fl
            meta[w, :, 16] = np.minimum(b + np.arange(128), nlp - 1).astype(np.int32)
            if cnt == 0:
                continue
            e0, e1 = starts[b], starts[b + cnt]
            ne = int(e1 - e0)
            assert ne <= slot_cap
            slot_src = np.zeros(slot_cap, dtype=np.int32)
            slot_q = np.zeros(slot_cap, dtype=np.int32)
            slot_rd = np.full(slot_cap, -1.0, dtype=np.float32)
            slot_src[:ne] = s_c[e0:e1]
            slot_q[:ne] = dl[e0:e1]
            slot_rd[:ne] = (dl[e0:e1] - b).astype(np.float32)
            slot_rd = slot_rd.view(np.int32)
            # slot s -> (partition s % 128, tile s // 128)
            meta[w, :, 0:tpw] = slot_src.reshape(tpw, 128).T
            meta[w, :, 32:32 + tpw] = slot_rd.reshape(tpw, 128).T

        x_loc_pad = np.zeros((nlp, 128), dtype=np.float32)
        x_loc_pad[:nl] = x[lo:lo + nl]
        xT_loc = np.ascontiguousarray(x_loc_pad.T)
        xbT_loc = xT_loc.copy()
        xbT_loc[:, :nl] += beff[:, None]

        im = dict(shared)
        im["xT_loc"] = xT_loc
        im["xbT_loc"] = np.ascontiguousarray(xbT_loc)
        im["meta_all"] = meta
        in_maps.append(im)
    return in_maps


# ---------------------------------------------------------------------------
# Device kernel
# ---------------------------------------------------------------------------

def build_kernel(cfg, n_real_total, phases="full", guard=True):
    """Build the Bass program. Returns finalized nc."""
    import concourse.bacc as bacc
    import concourse.tile as tile
    import concourse.mybir as mybir
    from concourse import bass
    from concourse.masks import make_identity

    dt = mybir.dt
    nlp = cfg["nlp"]; nfull = cfg["nfull"]; nw = cfg["nw"]
    slot_cap = cfg["slot_cap"]; trash0 = cfg["trash0"]
    tpw = slot_cap // 128
    nt_loc = nlp // 128
    nt_full = nfull // 128
    agg_rows = trash0 + nw * 128
    kvw = 257  # k(128) | v(128) | ones(1)
    inv_sqrt_d = 1.0 / math.sqrt(128.0)
    inv_n = 1.0 / float(n_real_total)

    nc = bacc.Bacc(None, target_bir_lowering=False, debug=False)

    # ---- I/O ----
    xT_full = nc.declare_dram_parameter("xT_full", [128, nfull], dt.float32, isOutput=False)
    xT_loc = nc.declare_dram_parameter("xT_loc", [128, nlp], dt.float32, isOutput=False)
    xbT_loc = nc.declare_dram_parameter("xbT_loc", [128, nlp], dt.float32, isOutput=False)
    meta_all = nc.declare_dram_parameter("meta_all", [nw, 128, 49], dt.int32, isOutput=False)
    Wkv = nc.declare_dram_parameter("Wkv", [128, 256], dt.float32, isOutput=False)
    Wq_ = nc.declare_dram_parameter("Wq_", [128, 128], dt.float32, isOutput=False)
    bkv_b = nc.declare_dram_parameter("bkv_b", [128, 256], dt.float32, isOutput=False)
    bq_b = nc.declare_dram_parameter("bq_b", [128, 128], dt.float32, isOutput=False)
    WsWO = nc.declare_dram_parameter("WsWO", [128, 128], dt.float32, isOutput=False)
    WO_ = nc.declare_dram_parameter("WO_", [128, 128], dt.float32, isOutput=False)
    W1_ = nc.declare_dram_parameter("W1_", [128, 256], dt.float32, isOutput=False)
    W2_ = nc.declare_dram_parameter("W2_", [256, 128], dt.float32, isOutput=False)
    bcols = nc.declare_dram_parameter("bcols", [128, 8], dt.float32, isOutput=False)
    yT_out = nc.declare_dram_parameter("yT_out", [128, nlp], dt.float32, isOutput=True)

    # ---- internal DRAM ----
    kv_tab = nc.dram_tensor("kv_tab", [nfull, kvw], dt.float32)
    q_tab = nc.dram_tensor("q_tab", [nlp, 128], dt.float32)
    agg_tab = nc.dram_tensor("agg_tab", [agg_rows, 129], dt.float32)
    st1_in = nc.dram_tensor("st1_in", [128, 2], dt.float32)
    st1_out = nc.dram_tensor("st1_out", [128, 2], dt.float32, addr_space="Shared")
    st2_in = nc.dram_tensor("st2_in", [128, 2], dt.float32)
    st2_out = nc.dram_tensor("st2_out", [128, 2], dt.float32, addr_space="Shared")

    rg = [list(range(cfg["nc"]))]

    with tile.TileContext(nc) as tc:
        with (
            tc.tile_pool(name="const", bufs=1) as constp,
            tc.tile_pool(name="w", bufs=1) as wp,
            tc.tile_pool(name="io", bufs=3) as iop,
            tc.tile_pool(name="kvout", bufs=3) as kvoutp,
            tc.tile_pool(name="gath", bufs=6) as gathp,
            tc.tile_pool(name="edge", bufs=4) as edgep,
            tc.tile_pool(name="small", bufs=4) as smallp,
            tc.tile_pool(name="p2", bufs=3) as p2p,
            tc.tile_pool(name="hold", bufs=1) as holdp,
            tc.tile_pool(name="psp", bufs=2, space="PSUM") as psp,
        ):
            # ---------------- constants ----------------
            iota_f = constp.tile([128, 128], dt.float32)
            nc.gpsimd.iota(iota_f[:], pattern=[[1, 128]], base=0,
                           channel_multiplier=0,
                           allow_small_or_imprecise_dtypes=True)
            ident = constp.tile([128, 128], dt.float32)
            make_identity(nc, ident[:])

            w_kv = wp.tile([128, 256], dt.float32)
            nc.sync.dma_start(w_kv[:], Wkv[:, :])
            w_q = wp.tile([128, 128], dt.float32)
            nc.sync.dma_start(w_q[:], Wq_[:, :])
            b_kv = wp.tile([128, 256], dt.float32)
            nc.sync.dma_start(b_kv[:], bkv_b[:, :])
            b_q = wp.tile([128, 128], dt.float32)
            nc.sync.dma_start(b_q[:], bq_b[:, :])
            w_swo = wp.tile([128, 128], dt.float32)
            nc.sync.dma_start(w_swo[:], WsWO[:, :])
            w_o = wp.tile([128, 128], dt.float32)
            nc.sync.dma_start(w_o[:], WO_[:, :])
            w_1 = wp.tile([128, 256], dt.float32)
            nc.sync.dma_start(w_1[:], W1_[:, :])
            w_2 = wp.tile([128, 256], dt.float32)  # [0:128]=W2a rows, [128:256] cols? no:
            # W2 is [256,128]; load as two [128,128] tiles side by side
            nc.sync.dma_start(w_2[:, 0:128], W2_[0:128, :])
            nc.sync.dma_start(w_2[:, 128:256], W2_[128:256, :])
            bc = wp.tile([128, 8], dt.float32)
            nc.sync.dma_start(bc[:], bcols[:, :])

            # ---------------- phase 0a: kv table (full) ----------------
            for t in range(nt_full):
                xt = iop.tile([128, 128], dt.float32, tag="xt")
                nc.sync.dma_start(xt[:], xT_full[:, t * 128:(t + 1) * 128])
                ps = psp.tile([128, 256], dt.float32, tag="psw")
                nc.tensor.matmul(ps[:], lhsT=xt[:], rhs=w_kv[:], start=True, stop=True)
                kvo = kvoutp.tile([128, kvw], dt.float32)
                nc.vector.tensor_tensor(
                    out=kvo[:, 0:256], in0=ps[:], in1=b_kv[:], op=mybir.AluOpType.add
                )
                nc.gpsimd.memset(kvo[:, 256:kvw], 1.0)
                nc.sync.dma_start(kv_tab[t * 128:(t + 1) * 128, :], kvo[:])

            # ---------------- phase 0b: q table (local) ----------------
            for t in range(nt_loc):
                xt = iop.tile([128, 128], dt.float32, tag="xt")
                nc.sync.dma_start(xt[:], xT_loc[:, t * 128:(t + 1) * 128])
                ps = psp.tile([128, 256], dt.float32, tag="psw")
                nc.tensor.matmul(ps[:, 0:128], lhsT=xt[:], rhs=w_q[:], start=True, stop=True)
                qo = kvoutp.tile([128, kvw], dt.float32, tag="qo")
                nc.vector.tensor_tensor(
                    out=qo[:, 0:128], in0=ps[:, 0:128], in1=b_q[:], op=mybir.AluOpType.add
                )
                nc.sync.dma_start(q_tab[t * 128:(t + 1) * 128, :], qo[:, 0:128])

            # ---------------- phase 1: edge windows ----------------
            gdump = constp.tile([128, nw], dt.float32)
            for w in (range(nw) if phases != "p0" else ()):
                meta = smallp.tile([128, 49], dt.int32, tag="meta")
                nc.sync.dma_start(meta[:], meta_all[w, :, :])
                qwin = gathp.tile([128, 128], dt.float32, tag="qwin")
                nc.gpsimd.indirect_dma_start(
                    out=qwin[:],
                    out_offset=None,
                    in_=q_tab[:, :],
                    in_offset=bass.IndirectOffsetOnAxis(ap=meta[:, 16:17], axis=0),
                    bounds_check=nlp - 1 if guard else None,
                    oob_is_err=False,
                )
                acc = psp.tile([128, 129], dt.float32, tag="psacc")  # [agg(128) | den(1)]
                for t in range(tpw):
                    kvg = gathp.tile([128, kvw], dt.float32, tag="kvg")
                    nc.gpsimd.indirect_dma_start(
                        out=kvg[:],
                        out_offset=None,
                        in_=kv_tab[:, :],
                        in_offset=bass.IndirectOffsetOnAxis(ap=meta[:, t:t + 1], axis=0),
                        bounds_check=nfull - 1 if guard else None,
                        oob_is_err=False,
                    )
                    if phases == "p1g":
                        if t == 0:
                            nc.vector.tensor_tensor(
                                out=gdump[:, w:w + 1], in0=kvg[:, 0:1],
                                in1=qwin[:, 0:1], op=mybir.AluOpType.add)
                        continue
                    onehot = edgep.tile([128, 128], dt.float32, tag="onehot")
                    nc.vector.tensor_scalar(
                        out=onehot[:],
                        in0=iota_f[:],
                        scalar1=meta[:, 32 + t:33 + t].bitcast(dt.float32),
                        scalar2=None,
                        op0=mybir.AluOpType.is_equal,
                    )
                    ohT_ps = psp.tile([128, 128], dt.float32, tag="pstr")
                    nc.tensor.transpose(ohT_ps[:], in_=onehot[:], identity=ident[:])
                    ohT = edgep.tile([128, 128], dt.float32, tag="ohT")
                    nc.scalar.copy(ohT[:], ohT_ps[:])
                    qdst_ps = psp.tile([128, 128], dt.float32, tag="psw")
                    nc.tensor.matmul(qdst_ps[:], lhsT=ohT[:], rhs=qwin[:],
                                     start=True, stop=True)
                    junk = edgep.tile([128, 128], dt.float32, tag="junk")
                    scol = smallp.tile([128, 1], dt.float32, tag="scol")
                    nc.vector.tensor_tensor(
                        out=junk[:], in0=qdst_ps[:], in1=kvg[:, 0:128],
                        op=mybir.AluOpType.mult,
                    )
                    nc.vector.reduce_sum(scol[:], junk[:], axis=mybir.AxisListType.X)
                    pcol = smallp.tile([128, 1], dt.float32, tag="pcol")
                    nc.scalar.activation(
                        pcol[:], scol[:], mybir.ActivationFunctionType.Exp,
                        scale=inv_sqrt_d,
                    )
                    scat = edgep.tile([128, 128], dt.float32, tag="scat")
                    nc.scalar.activation(
                        scat[:], onehot[:], mybir.ActivationFunctionType.Copy,
                        scale=pcol[:],
                    )
                    nc.tensor.matmul(
                        acc[:],
                        lhsT=scat[:],
                        rhs=kvg[:, 128:kvw],
                        start=(t == 0),
                        stop=(t == tpw - 1),
                    )
                if phases == "p1g":
                    continue
                flush = kvoutp.tile([128, 129], dt.float32, tag="flush")
                nc.scalar.copy(flush[:], acc[:])
                if phases == "p1ns":
                    nc.sync.dma_start(
                        agg_tab[trash0 + w * 128:trash0 + (w + 1) * 128, :], flush[:])
                else:
                    nc.gpsimd.indirect_dma_start(
                        out=agg_tab[:, :],
                        out_offset=bass.IndirectOffsetOnAxis(ap=meta[:, 48:49], axis=0),
                        in_=flush[:],
                        in_offset=None,
                        bounds_check=agg_rows - 1 if guard else None,
                        oob_is_err=False,
                    )

            # ---------------- phase 2a ----------------
            if phases in ("p0", "p0p1", "p1g", "p1ns"):
                # debug passthrough: dump agg rows (or q table) into yT_out
                for t in range(nt_loc):
                    dbg = p2p.tile([128, 129], dt.float32, tag="agg")
                    if phases == "p0p1":
                        nc.sync.dma_start(dbg[:], agg_tab[t * 128:(t + 1) * 128, :])
                    elif phases in ("p1g", "p1ns"):
                        nc.sync.dma_start(dbg[:, 0:128], q_tab[t * 128:(t + 1) * 128, :])
                    else:
                        nc.sync.dma_start(dbg[:, 0:128], q_tab[t * 128:(t + 1) * 128, :])
                    nc.sync.dma_start(yT_out[:, t * 128:(t + 1) * 128], dbg[:, 0:128])
            if phases == "full":
                h3hold = holdp.tile([128, nlp], dt.float32, tag="h3hold")
                h5hold = holdp.tile([128, nlp], dt.float32, tag="h5hold")
                sum1 = constp.tile([128, nt_loc], dt.float32)
                sq1 = constp.tile([128, nt_loc], dt.float32)
                for t in range(nt_loc):
                    agg = p2p.tile([128, 129], dt.float32, tag="agg")
                    nc.sync.dma_start(agg[:], agg_tab[t * 128:(t + 1) * 128, :])
                    dsafe = smallp.tile([128, 1], dt.float32, tag="dsafe")
                    nc.vector.tensor_scalar_max(dsafe[:], agg[:, 128:129], 1e-30)
                    rec = smallp.tile([128, 1], dt.float32, tag="rec")
                    nc.vector.reciprocal(rec[:], dsafe[:])
                    hat = p2p.tile([128, 128], dt.float32, tag="hat")
                    nc.scalar.activation(
                        hat[:], agg[:, 0:128], mybir.ActivationFunctionType.Copy,
                        scale=rec[:],
                    )
                    hatT_ps = psp.tile([128, 128], dt.float32, tag="pstr")
                    nc.tensor.transpose(hatT_ps[:], in_=hat[:], identity=ident[:])
                    hatT = p2p.tile([128, 128], dt.float32, tag="hatT")
                    nc.scalar.copy(hatT[:], hatT_ps[:])
                    xt = iop.tile([128, 128], dt.float32, tag="xt")
                    nc.sync.dma_start(xt[:], xT_loc[:, t * 128:(t + 1) * 128])
                    ps = psp.tile([128, 129], dt.float32, tag="psacc")
                    nc.tensor.matmul(ps[:, 0:128], lhsT=w_swo[:], rhs=xt[:], start=True, stop=False)
                    nc.tensor.matmul(ps[:, 0:128], lhsT=w_o[:], rhs=hatT[:], start=False, stop=True)
                    xbt = iop.tile([128, 128], dt.float32, tag="xbt")
                    nc.sync.dma_start(xbt[:], xbT_loc[:, t * 128:(t + 1) * 128])
                    h3 = h3hold[:, t * 128:(t + 1) * 128]
                    nc.vector.tensor_tensor(out=h3, in0=ps[:, 0:128], in1=xbt[:], op=mybir.AluOpType.add)
                    # stats
                    nc.vector.reduce_sum(sum1[:, t:t + 1], h3, axis=mybir.AxisListType.X)
                    h3sq = p2p.tile([128, 128], dt.float32, tag="h3sq")
                    nc.scalar.activation(h3sq[:], h3, mybir.ActivationFunctionType.Square)
                    nc.vector.reduce_sum(sq1[:, t:t + 1], h3sq[:], axis=mybir.AxisListType.X)

                # ---------------- AllReduce 1 ----------------
                st_sb = constp.tile([128, 2], dt.float32)
                nc.vector.reduce_sum(st_sb[:, 0:1], sum1[:], axis=mybir.AxisListType.X)
                nc.vector.reduce_sum(st_sb[:, 1:2], sq1[:], axis=mybir.AxisListType.X)
                nc.sync.dma_start(st1_in[:, :], st_sb[:])
                nc.gpsimd.collective_compute(
                    "AllReduce", mybir.AluOpType.add, replica_groups=rg,
                    ins=[st1_in[:, :].opt()], outs=[st1_out[:, :].opt()],
                )
                stg = constp.tile([128, 2], dt.float32)
                nc.sync.dma_start(stg[:], st1_out[:, :])
                s1c = constp.tile([128, 1], dt.float32)
                t1c = constp.tile([128, 1], dt.float32)
                _bn_coeffs(nc, mybir, smallp, stg, bc[:, 3:4], bc[:, 4:5], inv_n, s1c, t1c)

                # ---------------- phase 2b: BN1 -> FFN -> h5T ----------------
                sum2 = constp.tile([128, nt_loc], dt.float32)
                sq2 = constp.tile([128, nt_loc], dt.float32)
                for t in range(nt_loc):
                    bnh = p2p.tile([128, 128], dt.float32, tag="bnh")
                    nc.scalar.activation(
                        bnh[:], h3hold[:, t * 128:(t + 1) * 128],
                        mybir.ActivationFunctionType.Identity,
                        bias=t1c[:], scale=s1c[:],
                    )
                    if t == nt_loc - 1:
                        pad0 = (cfg["nl"] % 128) or 128
                        if pad0 < 128:
                            nc.gpsimd.memset(bnh[:, pad0:128], 0.0)
                    f1 = psp.tile([128, 256], dt.float32, tag="psw")
                    nc.tensor.matmul(f1[:, 0:128], lhsT=w_1[:, 0:128], rhs=bnh[:], start=True, stop=True)
                    nc.tensor.matmul(f1[:, 128:256], lhsT=w_1[:, 128:256], rhs=bnh[:], start=True, stop=True)
                    ra = p2p.tile([128, 256], dt.float32, tag="ra")
                    nc.scalar.activation(
                        ra[:, 0:128], f1[:, 0:128], mybir.ActivationFunctionType.Relu,
                        bias=bc[:, 0:1], scale=1.0,
                    )
                    nc.scalar.activation(
                        ra[:, 128:256], f1[:, 128:256], mybir.ActivationFunctionType.Relu,
                        bias=bc[:, 1:2], scale=1.0,
                    )
                    f2 = psp.tile([128, 129], dt.float32, tag="psacc")
                    nc.tensor.matmul(f2[:, 0:128], lhsT=w_2[:, 0:128], rhs=ra[:, 0:128], start=True, stop=False)
                    nc.tensor.matmul(f2[:, 0:128], lhsT=w_2[:, 128:256], rhs=ra[:, 128:256], start=False, stop=True)
                    f2b = p2p.tile([128, 128], dt.float32, tag="f2b")
                    nc.scalar.activation(
                        f2b[:], f2[:, 0:128], mybir.ActivationFunctionType.Identity,
                        bias=bc[:, 2:3], scale=1.0,
                    )
                    h5 = h5hold[:, t * 128:(t + 1) * 128]
                    nc.vector.tensor_tensor(out=h5, in0=f2b[:], in1=bnh[:], op=mybir.AluOpType.add)
                    if t == nt_loc - 1:
                        pad0 = (cfg["nl"] % 128) or 128
                        if pad0 < 128:
                            nc.gpsimd.memset(
                                h5hold[:, t * 128 + pad0:(t + 1) * 128], 0.0)
                    nc.vector.reduce_sum(sum2[:, t:t + 1], h5, axis=mybir.AxisListType.X)
                    h5sq = p2p.tile([128, 128], dt.float32, tag="h5sq")
                    nc.scalar.activation(h5sq[:], h5, mybir.ActivationFunctionType.Square)
                    nc.vector.reduce_sum(sq2[:, t:t + 1], h5sq[:], axis=mybir.AxisListType.X)

                # ---------------- AllReduce 2 ----------------
                st_sb2 = constp.tile([128, 2], dt.float32)
                nc.vector.reduce_sum(st_sb2[:, 0:1], sum2[:], axis=mybir.AxisListType.X)
                nc.vector.reduce_sum(st_sb2[:, 1:2], sq2[:], axis=mybir.AxisListType.X)
                nc.sync.dma_start(st2_in[:, :], st_sb2[:])
                nc.gpsimd.collective_compute(
                    "AllReduce", mybir.AluOpType.add, replica_groups=rg,
                    ins=[st2_in[:, :].opt()], outs=[st2_out[:, :].opt()],
                )
                stg2 = constp.tile([128, 2], dt.float32)
                nc.sync.dma_start(stg2[:], st2_out[:, :])
                s2c = constp.tile([128, 1], dt.float32)
                t2c = constp.tile([128, 1], dt.float32)
                _bn_coeffs(nc, mybir, smallp, stg2, bc[:, 5:6], bc[:, 6:7], inv_n, s2c, t2c)

                # ---------------- phase 2c: y = BN2(h5) ----------------
                for t in range(nt_loc):
                    yt = p2p.tile([128, 128], dt.float32, tag="yt")
                    nc.scalar.activation(
                        yt[:], h5hold[:, t * 128:(t + 1) * 128],
                        mybir.ActivationFunctionType.Identity,
                        bias=t2c[:], scale=s2c[:],
                    )
                    nc.sync.dma_start(yT_out[:, t * 128:(t + 1) * 128], yt[:])

    nc.finalize()
    return nc


def _bn_coeffs(nc, mybir, pool, stg, gcol, becol, inv_n, s_out, t_out):
    """From global (sum, sumsq) columns compute s = g*rstd, t = be - mu*s."""
    dt = mybir.dt
    mu = pool.tile([128, 1], dt.float32, tag="bn_mu")
    nc.scalar.activation(mu[:], stg[:, 0:1], mybir.ActivationFunctionType.Copy, scale=inv_n)
    e2 = pool.tile([128, 1], dt.float32, tag="bn_e2")
    nc.scalar.activation(e2[:], stg[:, 1:2], mybir.ActivationFunctionType.Copy, scale=inv_n)
    musq = pool.tile([128, 1], dt.float32, tag="bn_musq")
    nc.scalar.activation(musq[:], mu[:], mybir.ActivationFunctionType.Square)
    var = pool.tile([128, 1], dt.float32, tag="bn_var")
    nc.vector.tensor_tensor(out=var[:], in0=e2[:], in1=musq[:], op=mybir.AluOpType.subtract)
    varep = pool.tile([128, 1], dt.float32, tag="bn_varep")
    nc.vector.tensor_scalar_add(varep[:], var[:], EPS)
    sd = pool.tile([128, 1], dt.float32, tag="bn_sd")
    nc.scalar.activation(sd[:], varep[:], mybir.ActivationFunctionType.Sqrt)
    rstd = pool.tile([128, 1], dt.float32, tag="bn_rstd")
    nc.vector.reciprocal(rstd[:], sd[:])
    nc.vector.tensor_tensor(out=s_out[:], in0=gcol, in1=rstd[:], op=mybir.AluOpType.mult)
    mus = pool.tile([128, 1], dt.float32, tag="bn_mus")
    nc.vector.tensor_tensor(out=mus[:], in0=mu[:], in1=s_out[:], op=mybir.AluOpType.mult)
    nc.vector.tensor_tensor(out=t_out[:], in0=becol, in1=mus[:], op=mybir.AluOpType.subtract)


# ---------------------------------------------------------------------------
# Entry point
# ---------------------------------------------------------------------------

_CACHE = {}


def default_cfg():
    return {
        "n_nodes": N_NODES, "nc": NC, "nl": NL, "nlp": NLP, "nfull": NFULL,
        "slot_cap": SLOT_CAP, "nw": NW, "trash0": TRASH0,
    }


def kernel(x, edge_index, Wq, bq, Wk, bk, Wv, bv, Ws, bs, WO, bO,
           W1, b1, W2, b2, g1, be1, g2, be2):
    from concourse.bass_utils import run_bass_kernel_spmd

    cfg = default_cfg()
    weights = {
        "Wq": Wq, "bq": bq, "Wk": Wk, "bk": bk, "Wv": Wv, "bv": bv,
        "Ws": Ws, "bs": bs, "WO": WO, "bO": bO, "W1": W1, "b1": b1,
        "W2": W2, "b2": b2, "g1": g1, "be1": be1, "g2": g2, "be2": be2,
    }
    in_maps = host_prep(np.asarray(x), np.asarray(edge_index), weights, cfg)

    if "nc" not in _CACHE:
        _CACHE["nc"] = build_kernel(cfg, cfg["n_nodes"])
    nc = _CACHE["nc"]

    res = run_bass_kernel_spmd(nc, in_maps, core_ids=list(range(cfg["nc"])))
    outs = []
    for c in range(cfg["nc"]):
        yT = res.results[c]["yT_out"]
        outs.append(np.ascontiguousarray(yT.T[:cfg["nl"]]))
    return np.concatenate(outs, axis=0).astype(np.float32)



# revision 2
# speedup vs baseline: 2.3337x; 2.3337x over previous
"""TransformerConv GNN block (nn_Block_28192165331060) on 8 Trainium2 NeuronCores.

Design (v3):
  - Per-core input is only the core's own x slice (bf16, transposed), a
    packed edge-meta table and bf16 weights (~2.5 MB/core instead of the
    original ~34 MB/core): the full k/v table is built on device from the
    local x slice and AllGathered across the 8 cores (k bf16 | v bf16 |
    one bf16, 516-byte rows in an int8 tensor).
  - Destination nodes are grouped in FIXED windows of 112 (56 windows).
    All addressing is static: q for a window is a column slice of a
    precomputed SBUF-resident qT table; the window aggregate is
    normalized, transposed and written to an SBUF hold in-window. No
    indirect scatters, no q/agg DRAM round trips.
  - Per 128-edge tile: one indirect kv-row gather (the only SWDGE use),
    one-hot(rel_dst) mask, kT = PE-transpose(k), S = kT.T @ qT_win,
    per-edge score = masked row-sum of S (DVE), p = exp(score/sqrt(D)),
    scat = onehot * p (bf16), acc += scat.T @ [v | 1] in PSUM.
  - Up to LT=2 tiles per window hold edges whose SOURCE is core-local;
    they gather from kv_loc and execute while the AllGather is still in
    flight (engines drain queues in program order), accumulating into an
    SBUF side table that is added to the window aggregate.
  - Remote-tile counts per window are the max need over the 8 cores
    (shared program), computed from the actual edge data at build time.
  - Phase-2a tiles (h3 = x + skip + hat @ WO, BN1 stats) are interleaved
    into the window loop as soon as their hatT columns are final.
  - BatchNorm statistics are exact and global via two tiny AllReduces;
    the FFN runs in bf16 with f32 accumulation; all rounding together
    keeps max relative error ~2.7e-3 (gate is 2e-2).

  Only the shipped configuration (K_BF16=True, X_BF16=True) is supported;
  the False branches are historical.
"""

import math

import numpy as np

N_NODES = 50000
D = 128
NC = 8
NL = N_NODES // NC          # 6250 nodes per core
NLP = 6272                  # padded local nodes (49 * 128)
NT_LOC = NLP // 128         # 49 local node tiles
NFULL = NC * NLP            # 50176 padded full nodes
WIN = 112                   # dst nodes per window
NWIN = NLP // WIN           # 56 windows
LT = 2                      # local (pre-AllGather) tiles per window
PS_TR_BUFS = 2
PS_MM_BUFS = 2
PS_W_BUFS = 2
K_BF16 = True               # store k rows in bf16 (halves the kv AllGather)
X_BF16 = True               # ship x to the device in bf16 (halves the x transfer)
ROWB = 516 if K_BF16 else 772   # k | v bf16 256 | one bf16 2 | pad 2
EPS = 1e-5


# ---------------------------------------------------------------------------
# Host-side preprocessing
# ---------------------------------------------------------------------------

def host_prep(x, edge_index, weights, cfg):
    """Build per-core device input arrays. Returns (in_maps, tpw)."""
    n_nodes = cfg["n_nodes"]; nc_ = cfg["nc"]; nl = cfg["nl"]; nlp = cfg["nlp"]
    win = cfg["win"]; nwin = cfg["nwin"]

    x = np.asarray(x, dtype=np.float32)
    src = np.asarray(edge_index[0], dtype=np.int64)
    dst = np.asarray(edge_index[1], dtype=np.int64)

    W = {k: np.asarray(v, dtype=np.float32) for k, v in weights.items()}
    WsWO = (W["Ws"] @ W["WO"]).astype(np.float32)
    beff = (W["bs"] @ W["WO"] + W["bO"]).astype(np.float32)
    Wkv = np.concatenate([W["Wk"], W["Wv"]], axis=1).astype(np.float32)
    bkv_b = np.broadcast_to(
        np.concatenate([W["bk"], W["bv"]])[None, :], (128, 256)
    ).astype(np.float32).copy()

    # bias/affine columns
    bcols = np.zeros((128, 16), dtype=np.float32)
    bcols[:, 0] = W["b1"][0:128]
    bcols[:, 1] = W["b1"][128:256]
    bcols[:, 2] = W["b2"]
    bcols[:, 3] = W["g1"]
    bcols[:, 4] = W["be1"]
    bcols[:, 5] = W["g2"]
    bcols[:, 6] = W["be2"]
    bcols[:, 7] = beff
    bcols[:, 8] = W["bq"]

    # src index in the padded-chunk node space used by the gathered kv table
    src_pad = (src // nl) * nlp + (src % nl)

    # per-core edge lists grouped by fixed windows. Up to LT*128 edges with a
    # LOCAL source go into per-window local tiles (gathered from kv_loc while
    # the AllGather runs); the rest go into remote tiles whose count per
    # window is the max need over the 8 cores (the program is shared).
    owner = dst // nl
    srco = src // nl
    per_core = []
    need = np.ones(nwin, dtype=np.int64)
    for c in range(nc_):
        m = owner == c
        s_c = src_pad[m]
        loc_c = (srco[m] == c)
        dl = (dst[m] - c * nl).astype(np.int64)
        order = np.argsort(dl, kind="stable")
        s_c = s_c[order]
        loc_c = loc_c[order]
        dl = dl[order]
        deg = np.bincount(dl, minlength=nlp).astype(np.int64)
        starts = np.zeros(nlp + 1, dtype=np.int64)
        np.cumsum(deg, out=starts[1:])
        for w in range(nwin):
            e0, e1 = int(starts[w * win]), int(starts[(w + 1) * win])
            ne = e1 - e0
            cap = min(int(loc_c[e0:e1].sum()), LT * 128)
            need[w] = max(need[w], (ne - cap + 127) // 128)
        per_core.append((s_c, dl, starts, loc_c))

    twlist = tuple(int(t) for t in need)
    toff = np.zeros(nwin + 1, dtype=np.int64)
    np.cumsum(need, out=toff[1:])
    ttot = int(toff[-1])

    import jax.numpy as jnp
    tobf = lambda a: np.asarray(jnp.asarray(a, dtype=jnp.bfloat16))
    shared = {
        "Wkv": tobf(Wkv),
        "Wq_": tobf(W["Wq"]),
        "bkv_b": bkv_b,
        "WsWO": tobf(WsWO),
        "WO_": tobf(W["WO"]),
        "W1_": tobf(W["W1"]),
        "W2_": tobf(np.concatenate([W["W2"][0:128, :], W["W2"][128:256, :]],
                                   axis=1).astype(np.float32)),
        "bcols": bcols,
    }

    in_maps = []
    for c in range(nc_):
        s_c, dl, starts, loc_c = per_core[c]
        meta = np.full((128, ttot), 255, dtype=np.int32)
        metaL = np.full((128, nwin * LT), 255, dtype=np.int32)
        for w in range(nwin):
            e0, e1 = int(starts[w * win]), int(starts[(w + 1) * win])
            sw = s_c[e0:e1]
            lw = loc_c[e0:e1]
            rel = (dl[e0:e1] - w * win).astype(np.int32)
            li = np.where(lw)[0]
            cap = min(len(li), LT * 128)
            lsel = li[:cap]
            rmask = np.ones(len(sw), dtype=bool)
            rmask[lsel] = False
            # local tiles: row index within the local chunk
            lrow = (sw[lsel] % nlp).astype(np.int32)
            lpk = np.full(LT * 128, 255, dtype=np.int32)
            lpk[:cap] = (lrow << 8) | rel[lsel]
            metaL[:, w * LT:(w + 1) * LT] = lpk.reshape(LT, 128).T
            # remote tiles
            sr = sw[rmask]
            rr = rel[rmask]
            slots = int(need[w]) * 128
            assert len(sr) <= slots
            packed = np.full(slots, 255, dtype=np.int32)
            packed[:len(sr)] = (sr.astype(np.int32) << 8) | rr
            meta[:, int(toff[w]):int(toff[w + 1])] = packed.reshape(-1, 128).T

        lo = c * nl
        x_loc_pad = np.zeros((nlp, 128), dtype=np.float32)
        x_loc_pad[:nl] = x[lo:lo + nl]
        im = dict(shared)
        xT = np.ascontiguousarray(x_loc_pad.T)
        if X_BF16:
            import jax.numpy as jnp
            xT = np.asarray(jnp.asarray(xT, dtype=jnp.bfloat16))
        im["xT_loc"] = xT
        im["meta_all"] = meta
        im["metaL_all"] = metaL
        in_maps.append(im)
    return in_maps, twlist


# ---------------------------------------------------------------------------
# Device kernel
# ---------------------------------------------------------------------------

def build_kernel(cfg, twlist, n_real_total=N_NODES, guard=False):
    import concourse.bacc as bacc
    import concourse.tile as tile
    import concourse.mybir as mybir
    from concourse import bass
    from concourse.masks import make_identity

    dt = mybir.dt
    nlp = cfg["nlp"]; nfull = cfg["nfull"]; win = cfg["win"]; nwin = cfg["nwin"]
    nl = cfg["nl"]; nt_loc = nlp // 128
    assert len(twlist) == nwin
    toff = [0]
    for t in twlist:
        toff.append(toff[-1] + t)
    ttot = toff[-1]
    inv_sqrt_d = 1.0 / math.sqrt(128.0)
    inv_n = 1.0 / float(n_real_total)

    nc = bacc.Bacc(None, target_bir_lowering=False, debug=False)

    # ---- I/O ----
    xT_loc = nc.declare_dram_parameter(
        "xT_loc", [128, nlp], dt.bfloat16 if X_BF16 else dt.float32, isOutput=False)
    meta_all = nc.declare_dram_parameter("meta_all", [128, ttot], dt.int32, isOutput=False)
    metaL_all = nc.declare_dram_parameter("metaL_all", [128, nwin * LT], dt.int32, isOutput=False)
    Wkv = nc.declare_dram_parameter("Wkv", [128, 256], dt.bfloat16, isOutput=False)
    Wq_ = nc.declare_dram_parameter("Wq_", [128, 128], dt.bfloat16, isOutput=False)
    bkv_b = nc.declare_dram_parameter("bkv_b", [128, 256], dt.float32, isOutput=False)
    WsWO = nc.declare_dram_parameter("WsWO", [128, 128], dt.bfloat16, isOutput=False)
    WO_ = nc.declare_dram_parameter("WO_", [128, 128], dt.bfloat16, isOutput=False)
    W1_ = nc.declare_dram_parameter("W1_", [128, 256], dt.bfloat16, isOutput=False)
    W2_ = nc.declare_dram_parameter("W2_", [128, 256], dt.bfloat16, isOutput=False)
    bcols = nc.declare_dram_parameter("bcols", [128, 16], dt.float32, isOutput=False)
    yT_out = nc.declare_dram_parameter("yT_out", [128, nlp], dt.float32, isOutput=True)

    # ---- internal DRAM ----
    kv_loc = nc.dram_tensor("kv_loc", [nlp, ROWB], dt.int8)
    kv_tab = nc.dram_tensor("kv_tab", [nfull, ROWB], dt.int8, addr_space="Shared")
    st1_in = nc.dram_tensor("st1_in", [128, 2], dt.float32)
    st1_out = nc.dram_tensor("st1_out", [128, 2], dt.float32, addr_space="Shared")
    st2_in = nc.dram_tensor("st2_in", [128, 2], dt.float32)
    st2_out = nc.dram_tensor("st2_out", [128, 2], dt.float32, addr_space="Shared")

    rg = [list(range(cfg["nc"]))]

    with tile.TileContext(nc) as tc:
        with (
            tc.tile_pool(name="const", bufs=1) as constp,
            tc.tile_pool(name="w", bufs=1) as wp,
            tc.tile_pool(name="hold", bufs=1) as holdp,
            tc.tile_pool(name="kvb", bufs=3) as kvbp,
            tc.tile_pool(name="gath", bufs=18) as gathp,
            tc.tile_pool(name="edge", bufs=8) as edgep,
            tc.tile_pool(name="small", bufs=8) as smallp,
            tc.tile_pool(name="winp", bufs=3) as winp,
            tc.tile_pool(name="p2", bufs=2) as p2p,
            tc.tile_pool(name="ps_acc", bufs=2, space="PSUM") as ps_acc,
            tc.tile_pool(name="ps_tr", bufs=PS_TR_BUFS, space="PSUM") as ps_tr,
            tc.tile_pool(name="ps_mm", bufs=PS_MM_BUFS, space="PSUM") as ps_mm,
            tc.tile_pool(name="ps_w", bufs=PS_W_BUFS, space="PSUM") as ps_w,
        ):
            # ---------------- constants / weights ----------------
            iota_f = constp.tile([128, 128], dt.float32)
            nc.gpsimd.iota(iota_f[:], pattern=[[1, 128]], base=0,
                           channel_multiplier=0,
                           allow_small_or_imprecise_dtypes=True)
            ident = constp.tile([128, 128], dt.float32)
            make_identity(nc, ident[:])
            ident_bf = constp.tile([128, 128], dt.bfloat16)
            nc.scalar.copy(ident_bf[:], ident[:])

            def wload_bf(shape, param, nm):
                t = wp.tile(shape, dt.bfloat16, tag=nm)
                nc.sync.dma_start(t[:], param[:, :])
                return t

            w_kv = wload_bf([128, 256], Wkv, "w_kv")
            w_q = wload_bf([128, 128], Wq_, "w_q")
            w_swo = wload_bf([128, 128], WsWO, "w_swo")
            w_o = wload_bf([128, 128], WO_, "w_o")
            w_1 = wload_bf([128, 256], W1_, "w_1")
            w_2 = wload_bf([128, 256], W2_, "w_2")
            b_kv = wp.tile([128, 256], dt.float32)
            nc.sync.dma_start(b_kv[:], bkv_b[:, :])
            bc = wp.tile([128, 16], dt.float32)
            nc.sync.dma_start(bc[:], bcols[:, :])

            # SBUF-resident x (transposed, bf16), holds
            xsb = holdp.tile([128, nlp], dt.bfloat16, tag="xsb")
            nc.sync.dma_start(xsb[:], xT_loc[:, :])
            h3hold = holdp.tile([128, nlp], dt.float32, tag="h3hold")
            h5hold = holdp.tile([128, nlp], dt.float32, tag="h5hold")
            hatT_hold = holdp.tile([128, nlp], dt.bfloat16, tag="hatT")
            qT_hold = holdp.tile([128, nlp],
                                 dt.bfloat16 if K_BF16 else dt.float32, tag="qT")
            accL_hold = holdp.tile([128, 129 * nwin], dt.float32, tag="accL")

            # ---------------- phase 0a: local kv chunk ----------------
            for _ in range(3):
                o8 = kvbp.tile([128, ROWB], dt.int8, tag="out8")
                if K_BF16:
                    nc.gpsimd.memset(o8[:, 512:514].bitcast(dt.bfloat16), 1.0)
                    nc.gpsimd.memset(o8[:, 514:516], 0)
                else:
                    nc.gpsimd.memset(o8[:, 768:770].bitcast(dt.bfloat16), 1.0)
                    nc.gpsimd.memset(o8[:, 770:772], 0)
            for t in range(nt_loc):
                xt = xsb[:, t * 128:(t + 1) * 128]
                ps = ps_w.tile([128, 256], dt.float32, tag="psw")
                nc.tensor.matmul(ps[:], lhsT=xt, rhs=w_kv[:], start=True, stop=True)
                kvf = kvbp.tile([128, 256], dt.float32, tag="kvf")
                nc.vector.tensor_tensor(out=kvf[:], in0=ps[:], in1=b_kv[:],
                                        op=mybir.AluOpType.add)
                out8 = kvbp.tile([128, ROWB], dt.int8, tag="out8")
                if K_BF16:
                    nc.scalar.copy(out8[:, 0:256].bitcast(dt.bfloat16), kvf[:, 0:128])
                    nc.scalar.copy(out8[:, 256:512].bitcast(dt.bfloat16), kvf[:, 128:256])
                else:
                    nc.scalar.copy(out8[:, 0:512].bitcast(dt.float32), kvf[:, 0:128])
                    nc.scalar.copy(out8[:, 512:768].bitcast(dt.bfloat16), kvf[:, 128:256])
                nc.sync.dma_start(kv_loc[t * 128:(t + 1) * 128, :], out8[:])

            # ---------------- AllGather kv ----------------
            nc.gpsimd.collective_compute(
                "AllGather", mybir.AluOpType.bypass, replica_groups=rg,
                ins=[kv_loc[:, :].opt()], outs=[kv_tab[:, :].opt()],
            )

            # ---------------- phase 0b: skip-path + qT precompute (overlaps AG)
            for t in range(nt_loc):
                xt = xsb[:, t * 128:(t + 1) * 128]
                ps = ps_w.tile([128, 256], dt.float32, tag="psw")
                nc.tensor.matmul(ps[:, 0:128], lhsT=w_swo[:], rhs=xt,
                                 start=True, stop=True)
                nc.tensor.matmul(ps[:, 128:256], lhsT=w_q[:], rhs=xt,
                                 start=True, stop=True)
                tmp = kvbp.tile([128, 128], dt.float32, tag="sk")
                nc.scalar.activation(
                    tmp[:], ps[:, 0:128], mybir.ActivationFunctionType.Identity,
                    bias=bc[:, 7:8], scale=1.0)
                xf = kvbp.tile([128, 128], dt.float32, tag="xf")
                nc.scalar.copy(xf[:], xt)
                nc.vector.tensor_tensor(
                    out=h3hold[:, t * 128:(t + 1) * 128], in0=tmp[:], in1=xf[:],
                    op=mybir.AluOpType.add)
                nc.scalar.activation(
                    qT_hold[:, t * 128:(t + 1) * 128], ps[:, 128:256],
                    mybir.ActivationFunctionType.Identity,
                    bias=bc[:, 8:9], scale=1.0)
            # zero padded node columns so BN stats stay clean
            nc.gpsimd.memset(h3hold[:, nl:nlp], 0.0)

            # ---------------- phase 1: edge windows ----------------
            # bulk meta decode (overlaps the AllGather)
            msb = holdp.tile([128, ttot], dt.int32, tag="msb")
            nc.sync.dma_start(msb[:], meta_all[:, :])
            srcs = holdp.tile([128, ttot], dt.int32, tag="srcs")
            nc.vector.tensor_scalar(
                out=srcs[:], in0=msb[:], scalar1=8, scalar2=None,
                op0=mybir.AluOpType.logical_shift_right)
            nc.vector.tensor_scalar(
                out=msb[:], in0=msb[:], scalar1=255, scalar2=None,
                op0=mybir.AluOpType.bitwise_and)
            relf = holdp.tile([128, ttot], dt.float32, tag="relf")
            nc.scalar.copy(relf[:], msb[:])

            msbL = holdp.tile([128, nwin * LT], dt.int32, tag="msbL")
            nc.sync.dma_start(msbL[:], metaL_all[:, :])
            srcsL = holdp.tile([128, nwin * LT], dt.int32, tag="srcsL")
            nc.vector.tensor_scalar(
                out=srcsL[:], in0=msbL[:], scalar1=8, scalar2=None,
                op0=mybir.AluOpType.logical_shift_right)
            nc.vector.tensor_scalar(
                out=msbL[:], in0=msbL[:], scalar1=255, scalar2=None,
                op0=mybir.AluOpType.bitwise_and)
            relfL = holdp.tile([128, nwin * LT], dt.float32, tag="relfL")
            nc.scalar.copy(relfL[:], msbL[:])

            # pre-zero the two 'hat' buffers (partitions >= win stay 0 forever)
            for _ in range(2):
                h = winp.tile([128, 128], dt.float32, tag="hat", bufs=2)
                nc.gpsimd.memset(h[:], 0.0)

            def edge_tile(kvg_src, off_ap, rel_ap, qTw, acc, first, last):
                kvg = gathp.tile([128, ROWB], dt.int8, tag="kvg")
                nc.gpsimd.indirect_dma_start(
                    out=kvg[:], out_offset=None, in_=kvg_src,
                    in_offset=bass.IndirectOffsetOnAxis(ap=off_ap, axis=0),
                    bounds_check=None, oob_is_err=False,
                )
                onehot = edgep.tile([128, win], dt.float32, tag="onehot")
                nc.vector.tensor_scalar(
                    out=onehot[:], in0=iota_f[:, 0:win],
                    scalar1=rel_ap, scalar2=None,
                    op0=mybir.AluOpType.is_equal)
                if K_BF16:
                    kT_ps = ps_tr.tile([128, 128], dt.bfloat16, tag="ktr",
                                       padded_shape=[128, 256])
                    nc.tensor.transpose(kT_ps[:], in_=kvg[:, 0:256].bitcast(dt.bfloat16),
                                        identity=ident_bf[:])
                    kT = edgep.tile([128, 128], dt.bfloat16, tag="kT")
                else:
                    kT_ps = ps_tr.tile([128, 128], dt.float32, tag="ktr")
                    nc.tensor.transpose(kT_ps[:], in_=kvg[:, 0:512].bitcast(dt.float32),
                                        identity=ident[:])
                    kT = edgep.tile([128, 128], dt.float32, tag="kT")
                nc.scalar.copy(kT[:], kT_ps[:])
                s_ps = ps_mm.tile([128, 128], dt.float32, tag="mm")
                nc.tensor.matmul(s_ps[:, 0:win], lhsT=kT[:], rhs=qTw,
                                 start=True, stop=True)
                junk = edgep.tile([128, win], dt.float32, tag="junk")
                nc.vector.tensor_tensor(out=junk[:], in0=s_ps[:, 0:win], in1=onehot[:],
                                        op=mybir.AluOpType.mult)
                scol = smallp.tile([128, 1], dt.float32, tag="scol")
                nc.vector.reduce_sum(scol[:], junk[:], axis=mybir.AxisListType.X)
                pcol = smallp.tile([128, 1], dt.float32, tag="pcol")
                nc.scalar.activation(
                    pcol[:], scol[:], mybir.ActivationFunctionType.Exp,
                    scale=inv_sqrt_d)
                scat = edgep.tile([128, win], dt.bfloat16, tag="scat")
                nc.scalar.activation(
                    scat[:], onehot[:], mybir.ActivationFunctionType.Copy,
                    scale=pcol[:])
                nc.tensor.matmul(
                    acc[:], lhsT=scat[:],
                    rhs=kvg[:, 256:514].bitcast(dt.bfloat16) if K_BF16
                    else kvg[:, 512:770].bitcast(dt.bfloat16),
                    start=first, stop=last)

            # local pre-pass: gathers read kv_loc, so they run during the AG
            for w in range(nwin):
                qTw = qT_hold[:, w * win:(w + 1) * win]
                accL = ps_acc.tile([win, 129], dt.float32, tag="acc")
                for j in range(LT):
                    col = w * LT + j
                    edge_tile(kv_loc[:, :], srcsL[:, col:col + 1],
                              relfL[:, col:col + 1], qTw, accL,
                              j == 0, j == LT - 1)
                nc.scalar.copy(accL_hold[0:win, w * 129:(w + 1) * 129], accL[:])

            # remote windows (gathers wait on the AllGather); phase-2a tiles
            # are interleaved as soon as their hatT columns are complete
            sum1 = constp.tile([128, nt_loc], dt.float32)
            sq1 = constp.tile([128, nt_loc], dt.float32)

            def p2a_tile(t):
                ps = ps_mm.tile([128, 128], dt.float32, tag="mm")
                nc.tensor.matmul(ps[:, 0:128], lhsT=w_o[:],
                                 rhs=hatT_hold[:, t * 128:(t + 1) * 128],
                                 start=True, stop=True)
                h3 = h3hold[:, t * 128:(t + 1) * 128]
                nc.vector.tensor_tensor(out=h3, in0=ps[:, 0:128], in1=h3,
                                        op=mybir.AluOpType.add)
                nc.vector.reduce_sum(sum1[:, t:t + 1], h3, axis=mybir.AxisListType.X)
                h3sq = p2p.tile([128, 128], dt.float32, tag="h3sq")
                nc.scalar.activation(h3sq[:], h3, mybir.ActivationFunctionType.Square)
                nc.vector.reduce_sum(sq1[:, t:t + 1], h3sq[:], axis=mybir.AxisListType.X)

            p2a_done = 0
            for w in range(nwin):
                tw = twlist[w]
                qTw = qT_hold[:, w * win:(w + 1) * win]

                acc = ps_acc.tile([win, 129], dt.float32, tag="acc")
                for t in range(toff[w], toff[w] + tw):
                    edge_tile(kv_tab[:, :], srcs[:, t:t + 1], relf[:, t:t + 1],
                              qTw, acc, t == toff[w], t == toff[w] + tw - 1)

                asum = winp.tile([128, 129], dt.float32, tag="asum")
                nc.vector.tensor_tensor(
                    out=asum[0:win, :], in0=acc[:],
                    in1=accL_hold[0:win, w * 129:(w + 1) * 129],
                    op=mybir.AluOpType.add)
                dsafe = smallp.tile([128, 1], dt.float32, tag="dsafe")
                nc.vector.tensor_scalar_max(dsafe[0:win, :], asum[0:win, 128:129], 1e-30)
                rec = smallp.tile([128, 1], dt.float32, tag="rec")
                nc.vector.reciprocal(rec[0:win, :], dsafe[0:win, :])
                hat = winp.tile([128, 128], dt.float32, tag="hat", bufs=2)
                nc.scalar.activation(
                    hat[0:win, 0:128], asum[0:win, 0:128],
                    mybir.ActivationFunctionType.Copy, scale=rec[0:win, :])
                hatT_ps = ps_tr.tile([128, 128], dt.float32, tag="ktr")
                nc.tensor.transpose(hatT_ps[:], in_=hat[:], identity=ident[:])
                nc.scalar.copy(hatT_hold[:, w * win:(w + 1) * win],
                               hatT_ps[:, 0:win])
                while p2a_done < nt_loc and (p2a_done + 1) * 128 <= (w + 1) * win:
                    p2a_tile(p2a_done)
                    p2a_done += 1
            while p2a_done < nt_loc:
                p2a_tile(p2a_done)
                p2a_done += 1

            # ---------------- AllReduce 1 ----------------
            st_sb = constp.tile([128, 2], dt.float32)
            nc.vector.reduce_sum(st_sb[:, 0:1], sum1[:], axis=mybir.AxisListType.X)
            nc.vector.reduce_sum(st_sb[:, 1:2], sq1[:], axis=mybir.AxisListType.X)
            nc.sync.dma_start(st1_in[:, :], st_sb[:])
            nc.gpsimd.collective_compute(
                "AllReduce", mybir.AluOpType.add, replica_groups=rg,
                ins=[st1_in[:, :].opt()], outs=[st1_out[:, :].opt()],
            )
            stg = constp.tile([128, 2], dt.float32)
            nc.sync.dma_start(stg[:], st1_out[:, :])
            s1c = constp.tile([128, 1], dt.float32)
            t1c = constp.tile([128, 1], dt.float32)
            _bn_coeffs(nc, mybir, smallp, stg, bc[:, 3:4], bc[:, 4:5], inv_n, s1c, t1c)

            # ---------------- phase 2b: BN1 -> FFN -> h5 ----------------
            sum2 = constp.tile([128, nt_loc], dt.float32)
            sq2 = constp.tile([128, nt_loc], dt.float32)
            pad0 = (nl % 128) or 128
            for t in range(nt_loc):
                bnh = p2p.tile([128, 128], dt.bfloat16, tag="bnh")
                nc.scalar.activation(
                    bnh[:], h3hold[:, t * 128:(t + 1) * 128],
                    mybir.ActivationFunctionType.Identity,
                    bias=t1c[:], scale=s1c[:])
                bnf = p2p.tile([128, 128], dt.float32, tag="bnf")
                nc.vector.tensor_scalar(
                    out=bnf[:], in0=h3hold[:, t * 128:(t + 1) * 128],
                    scalar1=s1c[:], scalar2=t1c[:],
                    op0=mybir.AluOpType.mult, op1=mybir.AluOpType.add)
                if t == nt_loc - 1 and pad0 < 128:
                    nc.gpsimd.memset(bnh[:, pad0:128], 0.0)
                    nc.gpsimd.memset(bnf[:, pad0:128], 0.0)
                f1 = ps_w.tile([128, 256], dt.float32, tag="psw")
                nc.tensor.matmul(f1[:, 0:128], lhsT=w_1[:, 0:128], rhs=bnh[:],
                                 start=True, stop=True)
                nc.tensor.matmul(f1[:, 128:256], lhsT=w_1[:, 128:256], rhs=bnh[:],
                                 start=True, stop=True)
                ra = p2p.tile([128, 256], dt.bfloat16, tag="ra")
                nc.scalar.activation(
                    ra[:, 0:128], f1[:, 0:128], mybir.ActivationFunctionType.Relu,
                    bias=bc[:, 0:1], scale=1.0)
                nc.scalar.activation(
                    ra[:, 128:256], f1[:, 128:256], mybir.ActivationFunctionType.Relu,
                    bias=bc[:, 1:2], scale=1.0)
                f2 = ps_mm.tile([128, 128], dt.float32, tag="mm")
                nc.tensor.matmul(f2[:], lhsT=w_2[:, 0:128],
                                 rhs=ra[:, 0:128], start=True, stop=False)
                nc.tensor.matmul(f2[:], lhsT=w_2[:, 128:256],
                                 rhs=ra[:, 128:256], start=False, stop=True)
                f2b = p2p.tile([128, 128], dt.float32, tag="f2b")
                nc.scalar.activation(
                    f2b[:], f2[:], mybir.ActivationFunctionType.Identity,
                    bias=bc[:, 2:3], scale=1.0)
                h5 = h5hold[:, t * 128:(t + 1) * 128]
                nc.vector.tensor_tensor(out=h5, in0=f2b[:], in1=bnf[:],
                                        op=mybir.AluOpType.add)
                if t == nt_loc - 1 and pad0 < 128:
                    nc.gpsimd.memset(h5hold[:, t * 128 + pad0:(t + 1) * 128], 0.0)
                nc.vector.reduce_sum(sum2[:, t:t + 1], h5, axis=mybir.AxisListType.X)
                h5sq = p2p.tile([128, 128], dt.float32, tag="h3sq")
                nc.scalar.activation(h5sq[:], h5, mybir.ActivationFunctionType.Square)
                nc.vector.reduce_sum(sq2[:, t:t + 1], h5sq[:], axis=mybir.AxisListType.X)

            # ---------------- AllReduce 2 ----------------
            st_sb2 = constp.tile([128, 2], dt.float32)
            nc.vector.reduce_sum(st_sb2[:, 0:1], sum2[:], axis=mybir.AxisListType.X)
            nc.vector.reduce_sum(st_sb2[:, 1:2], sq2[:], axis=mybir.AxisListType.X)
            nc.sync.dma_start(st2_in[:, :], st_sb2[:])
            nc.gpsimd.collective_compute(
                "AllReduce", mybir.AluOpType.add, replica_groups=rg,
                ins=[st2_in[:, :].opt()], outs=[st2_out[:, :].opt()],
            )
            stg2 = constp.tile([128, 2], dt.float32)
            nc.sync.dma_start(stg2[:], st2_out[:, :])
            s2c = constp.tile([128, 1], dt.float32)
            t2c = constp.tile([128, 1], dt.float32)
            _bn_coeffs(nc, mybir, smallp, stg2, bc[:, 5:6], bc[:, 6:7], inv_n, s2c, t2c)

            # ---------------- phase 2c: y = BN2(h5) ----------------
            for t0 in range(0, nt_loc, 4):
                n = min(4, nt_loc - t0) * 128
                yt = p2p.tile([128, 512], dt.float32, tag="yt")
                nc.scalar.activation(
                    yt[:, 0:n], h5hold[:, t0 * 128:t0 * 128 + n],
                    mybir.ActivationFunctionType.Identity,
                    bias=t2c[:], scale=s2c[:])
                nc.sync.dma_start(yT_out[:, t0 * 128:t0 * 128 + n], yt[:, 0:n])

    nc.finalize()
    return nc


def _bn_coeffs(nc, mybir, pool, stg, gcol, becol, inv_n, s_out, t_out):
    """From global (sum, sumsq) columns compute s = g*rstd, t = be - mu*s."""
    dt = mybir.dt
    mu = pool.tile([128, 1], dt.float32, tag="bn_mu")
    nc.scalar.activation(mu[:], stg[:, 0:1], mybir.ActivationFunctionType.Copy, scale=inv_n)
    e2 = pool.tile([128, 1], dt.float32, tag="bn_e2")
    nc.scalar.activation(e2[:], stg[:, 1:2], mybir.ActivationFunctionType.Copy, scale=inv_n)
    musq = pool.tile([128, 1], dt.float32, tag="bn_musq")
    nc.scalar.activation(musq[:], mu[:], mybir.ActivationFunctionType.Square)
    var = pool.tile([128, 1], dt.float32, tag="bn_var")
    nc.vector.tensor_tensor(out=var[:], in0=e2[:], in1=musq[:], op=mybir.AluOpType.subtract)
    varep = pool.tile([128, 1], dt.float32, tag="bn_varep")
    nc.vector.tensor_scalar_add(varep[:], var[:], EPS)
    sd = pool.tile([128, 1], dt.float32, tag="bn_sd")
    nc.scalar.activation(sd[:], varep[:], mybir.ActivationFunctionType.Sqrt)
    rstd = pool.tile([128, 1], dt.float32, tag="bn_rstd")
    nc.vector.reciprocal(rstd[:], sd[:])
    nc.vector.tensor_tensor(out=s_out[:], in0=gcol, in1=rstd[:], op=mybir.AluOpType.mult)
    mus = pool.tile([128, 1], dt.float32, tag="bn_mus")
    nc.vector.tensor_tensor(out=mus[:], in0=mu[:], in1=s_out[:], op=mybir.AluOpType.mult)
    nc.vector.tensor_tensor(out=t_out[:], in0=becol, in1=mus[:], op=mybir.AluOpType.subtract)


# ---------------------------------------------------------------------------
# Entry point
# ---------------------------------------------------------------------------

_CACHE = {}


def default_cfg():
    return {
        "n_nodes": N_NODES, "nc": NC, "nl": NL, "nlp": NLP, "nfull": NFULL,
        "win": WIN, "nwin": NWIN,
    }


def kernel(x, edge_index, Wq, bq, Wk, bk, Wv, bv, Ws, bs, WO, bO,
           W1, b1, W2, b2, g1, be1, g2, be2):
    from concourse.bass_utils import run_bass_kernel_spmd

    cfg = default_cfg()
    weights = {
        "Wq": Wq, "bq": bq, "Wk": Wk, "bk": bk, "Wv": Wv, "bv": bv,
        "Ws": Ws, "bs": bs, "WO": WO, "bO": bO, "W1": W1, "b1": b1,
        "W2": W2, "b2": b2, "g1": g1, "be1": be1, "g2": g2, "be2": be2,
    }
    in_maps, twlist = host_prep(np.asarray(x), np.asarray(edge_index), weights, cfg)

    if ("nc", twlist) not in _CACHE:
        _CACHE[("nc", twlist)] = build_kernel(cfg, twlist)
    nc = _CACHE[("nc", twlist)]

    res = run_bass_kernel_spmd(nc, in_maps, core_ids=list(range(cfg["nc"])))
    outs = []
    for c in range(cfg["nc"]):
        yT = res.results[c]["yT_out"]
        outs.append(np.ascontiguousarray(yT.T[:cfg["nl"]]))
    return np.concatenate(outs, axis=0).astype(np.float32)
